# revision 3
# baseline (speedup 1.0000x reference)
"""BinaryConv2d (3x3, stride 1, pad 1) on 8 TRN2 NeuronCores.

Data-parallel: batch 32 sharded 4-per-core; weight/bias replicated.

Algorithm: 1-D Winograd F(4,3) along H (upgraded from the earlier F(2,3)
version: 4.5 matmul-widths per output row instead of 6 -> 1.33x less PE
work, and PE is the bottleneck). For each tile row t (4 output rows) the
conv needs 6 H-transformed input slices built from xs rows 4t..4t+5:
  U0 = 4(e0-e2) + (e4-e2)      U1 = (e3+e4) - 4(e1+e2)
  U2 = 4(e1-e2) + (e4-e3)      U3 = 2(e3-e1) + (e4-e2)
  U4 = (e4-e2) - 2(e3-e1)      U5 = (e5-e3) - 4(e3-e1)
with an integer-scaled weight transform G' = diag(4,6,6,24,24,1)G whose
entries are integers in [-7,7] -- exact in fp8-e4m3, so the weight
stream stays fp8 (FWL 4x weight load). The 1/4,1/6,1/24 factors fold
into the inverse transform, applied once per V-term by ACT/DVE scaled
ops. U is stored fp16 (not bf16): the 4x/5x B^T coefficients push |U|
to ~35 and bf16's 8-bit mantissa would cost 1.2e-2 rel error; fp16
lands at ~2e-3 (measured against the f32 reference).

Per (group=7 tile rows, half of 256 out-channels): 18 matmuls (6 terms x
3 width taps, center tap full width N=392 with start=True, shifted taps
into column-shifted PSUM windows at N=385). Inverse transform per
group-half (V terms arrive pre-scaled by s_k):
  P6=V1/6+V2/6  M6=V1/6-V2/6  Q24=V3/24+V4/24  R24=V3/24-V4/24
  y0 = V0/4+b + P6 + Q24        y1 = M6 + (2 R24 + b)
  y2 = P6 + (4 Q24 + b)         y3 = M6 + (8 R24 + b) + V5
honoring the one-PSUM-operand-per-op limit: ACT makes the scaled SBUF
copies c0..c4 (reading PSUM), DVE does the two-tensor combines.

Output is written as fp16 (halves the 12.8MB/core output DMA; host
upcasts to f32; measured rel err 2.1e-3, 10x inside the 2e-2 budget).

Engine split: PE 288 real matmuls (~47us floor); GpSimd the 8 transform
subexpressions per image (staged 2 images ahead) + weight/bias DMA
issues; ACT the 5 transform scale ops + the 5 PSUM->SBUF scaled copies
per group-half + edge memsets; DVE the 6 U finals per image + the 11
remaining combine ops per group-half; Sync all image/output DMAs.
PE warmup dummy matmuls bridge the preamble+fill window so the HAM
clock-gate (K=4/8 cold throttle) lifts before the real stream.
"""

import numpy as np
from contextlib import ExitStack

import concourse.bass as bass
import concourse.bacc as bacc
import concourse.mybir as mybir
import concourse.tile as tile
from concourse.bass_utils import run_bass_kernel_spmd

N_CORES = 8
N_BATCH = 32
N_PER_CORE = N_BATCH // N_CORES  # 4
C_IN = 128
C_OUT = 256
H = W = 56
HP = H + 2           # zero-padded height (in xs)
T_IMG = H // 4       # 14 tile rows per image (4 output rows each)
T_GRP = 7            # tile rows per matmul group
NGRP = T_IMG // T_GRP  # 2 groups -> 28 output rows each

f32 = mybir.dt.float32
bf16 = mybir.dt.bfloat16
f16 = mybir.dt.float16
fp8 = mybir.dt.float8e4
AF = mybir.ActivationFunctionType
ALU = mybir.AluOpType

# matmul term order: k=0 (needed by y0 only) next-to-last, k=5 (y3 only)
# last, so the y1/y2 chains complete during the k=0/5 matmuls and only
# the short y0/y3 tails trail the stream.
K_ORDER = (1, 2, 3, 4, 0, 5)


def build_program() -> bass.Bass:
    nc = bacc.Bacc("TRN2", target_bir_lowering=False, debug=False)
    x = nc.dram_tensor("x", [N_PER_CORE, C_IN, H, W], bf16, kind="ExternalInput")
    # wt[half, i, kk, dw, o]: host-transformed integer Winograd weights
    # (kk indexes K_ORDER), entries in [-7,7] -> fp8-e4m3 exact.
    wt = nc.dram_tensor("wt", [2, C_IN, 6, 3, 128], fp8, kind="ExternalInput")
    b = nc.dram_tensor("b", [C_OUT], f32, kind="ExternalInput")
    y = nc.dram_tensor("y", [N_PER_CORE, C_OUT, H, W], f16, kind="ExternalOutput")

    with tile.TileContext(nc) as tc, ExitStack() as ctx:
        singles = ctx.enter_context(tc.tile_pool(name="singles", bufs=1))
        xsp = ctx.enter_context(tc.tile_pool(name="xsp", bufs=4))
        up = ctx.enter_context(tc.tile_pool(name="up", bufs=3))
        itp = ctx.enter_context(tc.tile_pool(name="itp", bufs=2))
        psum_mm = ctx.enter_context(
            tc.tile_pool(name="psum_mm", bufs=8, space="PSUM")
        )
        tdp = ctx.enter_context(tc.tile_pool(name="tdp", bufs=2))
        obp = ctx.enter_context(tc.tile_pool(name="obp", bufs=4))

        # warmup weight tile first: its GpSimd memset is the earliest
        # producer any PE work can gate on
        warm_w = singles.tile([128, 128], bf16)
        nc.gpsimd.memset(warm_w, 0.0)

        wtile = singles.tile([128, 2, 6, 3, 128], fp8, name="wt")
        bsb = singles.tile([128, 2], f32)

        def transform_piece(xs, U, t0, t1, sub_eng, fin_eng, scl_eng):
            """B^T transform for tile rows [t0, t1): 8 two-tensor
            subexpressions (sub_eng), 5 scale ops (scl_eng: engine.mul
            style (out, in, mult)), 6 U finals (fin_eng). Intermediates
            fp16 (one extra rounding, ~2^-11 rel -- measured total
            2.1e-3)."""
            nT = t1 - t0

            def e(j):  # xs rows 4t+j for t in [t0, t1)
                lo = 4 * t0 + j
                return xs[:, lo:lo + 4 * (nT - 1) + 1:4, :]

            def it(tag):
                return itp.tile([128, nT, 56], f16, name=tag,
                                tag=f"{tag}_{nT}_{t0 % 2}")

            s1, d1, s2, d2 = it("s1"), it("d1"), it("s2"), it("d2")
            q, r, d0, d5 = it("q"), it("r"), it("d0"), it("d5")
            t4s1, t4d1, t2r = it("t4s1"), it("t4d1"), it("t2r")
            t4d0, t4r = it("t4d0"), it("t4r")

            sub_eng.tensor_add(s1, e(1), e(2))
            sub_eng.tensor_sub(d1, e(1), e(2))
            sub_eng.tensor_add(s2, e(3), e(4))
            sub_eng.tensor_sub(d2, e(4), e(3))
            sub_eng.tensor_sub(q, e(4), e(2))
            sub_eng.tensor_sub(r, e(3), e(1))
            sub_eng.tensor_sub(d0, e(0), e(2))
            sub_eng.tensor_sub(d5, e(5), e(3))

            for dst, src, m in ((t4s1, s1, 4.0), (t4d1, d1, 4.0),
                                (t2r, r, 2.0), (t4d0, d0, 4.0),
                                (t4r, r, 4.0)):
                scl_eng(dst, src, m)

            rr = slice(t0, t1)
            fin_eng.tensor_sub(U[1][:, rr, :], s2, t4s1)
            fin_eng.tensor_add(U[2][:, rr, :], t4d1, d2)
            fin_eng.tensor_add(U[3][:, rr, :], t2r, q)
            fin_eng.tensor_sub(U[4][:, rr, :], q, t2r)
            fin_eng.tensor_add(U[0][:, rr, :], t4d0, q)
            fin_eng.tensor_sub(U[5][:, rr, :], d5, t4r)

        def new_U():
            return [up.tile([128, T_IMG, W], f16, name=f"u{k}", tag=f"u{k}")
                    for k in range(6)]

        def act_mul(dst, src, m):
            nc.scalar.activation(dst, src, AF.Copy, scale=m)

        def gps_mul(dst, src, m):
            nc.gpsimd.tensor_scalar_mul(dst, src, m)

        def stage_image(n):
            """One contiguous DMA into the H-padded slab + edge memsets +
            the transform (GpSimd subexprs, ACT scales, DVE finals;
            staged 2 images ahead of consumption)."""
            xs = xsp.tile([128, HP, W], bf16, name="xs")
            nc.sync.dma_start(out=xs[:, 1:1 + H, :], in_=x.ap()[n])
            nc.scalar.memzero(xs[:, 0, :])
            nc.scalar.memzero(xs[:, HP - 1, :])
            U = new_U()
            transform_piece(xs, U, 0, T_IMG, nc.gpsimd, nc.vector, act_mul)
            return U

        def stage_image0():
            """Image 0 rides the critical path: 2 row-chunk DMAs (the
            first group's transform gates on ~550KB, not 802KB) and
            per-group transform pieces; weights/bias issue from the
            (idle) GpSimd queue in parallel, (half0, kk0) first so MM#1
            gates on a 49KB transfer."""
            xs = xsp.tile([128, HP, W], bf16, name="xs")
            nc.gpsimd.dma_start(out=wtile[:, 0, 0], in_=wt.ap()[0][:, 0])
            nc.gpsimd.dma_start(out=wtile[:, 0, 1:6], in_=wt.ap()[0][:, 1:6])
            nc.gpsimd.dma_start(out=wtile[:, 1], in_=wt.ap()[1])
            nc.gpsimd.dma_start(
                out=bsb, in_=b.ap().rearrange("(h o) -> o h", h=2))
            # group 0 (tile rows 0..6) reads xs rows 0..30 = x rows 0..29
            nc.sync.dma_start(out=xs[:, 1:31, :], in_=x.ap()[0, :, 0:30, :])
            nc.sync.dma_start(out=xs[:, 31:57, :], in_=x.ap()[0, :, 30:56, :])
            nc.scalar.memzero(xs[:, 0, :])
            nc.scalar.memzero(xs[:, HP - 1, :])
            U = new_U()
            # both pieces on DVE/ACT (fast, ahead of the combine
            # backlog); GpSimd is busy issuing the weight DMAs
            transform_piece(xs, U, 0, T_GRP, nc.vector, nc.vector, act_mul)
            transform_piece(xs, U, T_GRP, T_IMG, nc.gpsimd, nc.vector,
                            act_mul)
            return U

        # ---- PE warmup: bridge the pipeline-fill window (preamble + first
        # image DMA + first transform piece) with dummy matmuls so the HAM
        # clock-gate lifts before the real stream starts.
        wp = psum_mm.tile([128, 128], f32, tag="ps")
        NWARM = 36
        for k in range(NWARM):
            nc.tensor.matmul(wp, lhsT=warm_w, rhs=warm_w,
                             start=(k == 0), stop=(k == NWARM - 1))

        def do_group(n, U, g, half, split_tail=False):
            """28 output rows (tile rows 7g..7g+6) of image n, one half."""
            h0 = 4 * T_GRP * g
            r = slice(T_GRP * g, T_GRP * (g + 1))
            V = {}
            for kk, k in enumerate(K_ORDER):
                ps = psum_mm.tile([128, T_GRP, W], f32, name=f"v{k}",
                                  tag="ps")
                lt = wtile[:, half, kk]
                # center tap first at full width (sets has_written), then
                # the shifted taps accumulate into partial column windows
                nc.tensor.matmul(ps, lhsT=lt[:, 1], rhs=U[k][:, r, :],
                                 start=True, stop=False)
                nc.tensor.matmul(ps[:, :, 1:W], lhsT=lt[:, 0],
                                 rhs=U[k][:, r, 0:W - 1],
                                 start=False, stop=False)
                nc.tensor.matmul(ps[:, :, 0:W - 1], lhsT=lt[:, 2],
                                 rhs=U[k][:, r, 1:W],
                                 start=False, stop=True)
                V[k] = ps

            ob = obp.tile([128, T_GRP, 4, W], f16, name="ob", tag="ob")
            bias = bsb[:, half:half + 1]

            def td(tag):
                return tdp.tile([128, T_GRP, W], f32, name=tag, tag=tag)

            c1, c2, c3, c4, c0 = td("c1"), td("c2"), td("c3"), td("c4"), td("c0")
            P6, M6, Q24, R24 = td("P6"), td("M6"), td("Q24"), td("R24")
            tt, t2 = td("t"), td("t2")
            R12b, Q6b, R3b = td("R12b"), td("Q6b"), td("R3b")

            C6, C24 = 1.0 / 6.0, 1.0 / 24.0

            def combine(rlo, rhi):
                s = slice(rlo, rhi)
                # ACT: scaled PSUM->SBUF copies (the only engine besides
                # DVE that reads PSUM; one PSUM operand per op)
                nc.scalar.activation(c1[:, s], V[1][:, s], AF.Copy, scale=C6)
                nc.scalar.activation(c2[:, s], V[2][:, s], AF.Copy, scale=C6)
                nc.vector.tensor_add(P6[:, s], c1[:, s], c2[:, s])
                nc.vector.tensor_sub(M6[:, s], c1[:, s], c2[:, s])
                nc.scalar.activation(c3[:, s], V[3][:, s], AF.Copy, scale=C24)
                nc.scalar.activation(c4[:, s], V[4][:, s], AF.Copy, scale=C24)
                nc.vector.tensor_add(Q24[:, s], c3[:, s], c4[:, s])
                nc.vector.tensor_sub(R24[:, s], c3[:, s], c4[:, s])
                # scale+bias on DVE (scalar2 is a per-partition AP)
                nc.vector.tensor_scalar(out=R12b[:, s], in0=R24[:, s],
                                        scalar1=2.0, scalar2=bias,
                                        op0=ALU.mult, op1=ALU.add)
                nc.vector.tensor_scalar(out=Q6b[:, s], in0=Q24[:, s],
                                        scalar1=4.0, scalar2=bias,
                                        op0=ALU.mult, op1=ALU.add)
                nc.vector.tensor_scalar(out=R3b[:, s], in0=R24[:, s],
                                        scalar1=8.0, scalar2=bias,
                                        op0=ALU.mult, op1=ALU.add)
                nc.vector.tensor_add(ob[:, s, 1, :], M6[:, s], R12b[:, s])
                nc.vector.tensor_add(ob[:, s, 2, :], P6[:, s], Q6b[:, s])
                nc.vector.tensor_add(t2[:, s], M6[:, s], R3b[:, s])
                # y0 = V0/4 + b + P6 + Q24; y3 = t2 + V5
                nc.scalar.activation(c0[:, s], V[0][:, s], AF.Identity,
                                     bias=bias, scale=0.25)
                nc.vector.tensor_add(tt[:, s], c0[:, s], P6[:, s])
                nc.vector.tensor_add(ob[:, s, 0, :], tt[:, s], Q24[:, s])
                nc.vector.tensor_add(ob[:, s, 3, :], t2[:, s], V[5][:, s])

            ych = y.ap()[n, half * 128:(half + 1) * 128]
            if split_tail:
                # tail: two row-blocks so the first block's DMA overlaps
                # the second block's combine ops
                for rlo, rhi in ((0, 4), (4, T_GRP)):
                    combine(rlo, rhi)
                    nc.sync.dma_start(
                        out=ych[:, h0 + 4 * rlo:h0 + 4 * rhi, :],
                        in_=ob[:, rlo:rhi],
                    )
            else:
                combine(0, T_GRP)
                nc.sync.dma_start(out=ych[:, h0:h0 + 4 * T_GRP, :], in_=ob)

        # software pipeline: staged two images ahead so GpSimd's slower
        # transform rate never gates the PE
        Us = [stage_image0(), stage_image(1), stage_image(2)]
        for n in range(N_PER_CORE):
            if n + 3 < N_PER_CORE:
                Us.append(stage_image(n + 3))
            for g in range(NGRP):
                for half in range(2):
                    last = (n == N_PER_CORE - 1 and g == NGRP - 1
                            and half == 1)
                    do_group(n, Us[n], g, half, split_tail=last)
    nc.compile()
    return nc


# integer-scaled F(4,3) weight transform: diag(4,6,6,24,24,1) @ G
_GI = np.array([[1, 0, 0], [-1, -1, -1], [-1, 1, -1], [1, 2, 4],
                [1, -2, 4], [0, 0, 1]], dtype=np.float32)


def host_weight_layout(weight: np.ndarray) -> np.ndarray:
    """[256, 128, 3, 3] -> binarize, integer-G transform along dh,
    layout [half, i, kk, dw, o] = [2, 128, 6, 3, 128] fp8-e4m3
    (entries are integers in [-7, 7]: e4m3-exact)."""
    import ml_dtypes
    wc = np.clip(weight.astype(np.float32), -1.0, 1.0)
    wbin = np.where(wc >= 0, 1.0, -1.0).astype(np.float32)
    wtr = np.einsum("kd,oidw->koiw", _GI, wbin)    # [k, o, i, dw]
    wtr = wtr[list(K_ORDER)]                       # [kk, o, i, dw]
    w5 = wtr.reshape(6, 2, 128, C_IN, 3)           # [kk, half, oo, i, dw]
    w6 = w5.transpose(1, 3, 0, 4, 2)               # [half, i, kk, dw, oo]
    return np.ascontiguousarray(w6).astype(ml_dtypes.float8_e4m3fn)


def run(x, weight, bias, trace=False):
    """Returns (out [32,256,56,56] f32, BassKernelResults)."""
    import ml_dtypes
    nc = build_program()
    xb = np.asarray(x, dtype=np.float32).astype(ml_dtypes.bfloat16)
    wtr = host_weight_layout(np.asarray(weight))
    bias = np.ascontiguousarray(np.asarray(bias), dtype=np.float32)
    in_maps = [
        {
            "x": xb[i * N_PER_CORE:(i + 1) * N_PER_CORE],
            "wt": wtr,
            "b": bias,
        }
        for i in range(N_CORES)
    ]
    res = run_bass_kernel_spmd(
        nc, in_maps, core_ids=list(range(N_CORES)), trace=trace
    )
    out = np.concatenate([r["y"] for r in res.results], axis=0)
    return out.astype(np.float32), res


def kernel(x, weight, bias):
    out, _ = run(x, weight, bias)
    return out


# revision 7
# speedup vs baseline: 1.9731x; 1.9731x over previous
"""BinaryConv2d (3x3, stride 1, pad 1) on 8 TRN2 NeuronCores.

Data-parallel: batch 32 sharded 4-per-core; weight/bias replicated.

Algorithm: 1-D Winograd F(2,3) along H. For each pair of output rows
(tile row t) the conv needs 4 H-transformed input rows
  U0 = x[2t-1] - x[2t+1]   U1 = x[2t] + x[2t+1]
  U2 = x[2t+1] - x[2t]     U3 = x[2t] - x[2t+2]
and 4 transformed weight sets Wt[a][o,i,dw] = sum_dh G[a,dh] w[o,i,dh,dw]
(entries +-0.5/+-1.5/+-1, bf16-exact for binarized weights). Then
  V[a] = sum_dw Wt[a][:,:,dw] @ U[a] shifted by dw   (3 matmuls, PSUM)
  y[2t]   = V0 + V1 + V2 + bias
  y[2t+1] = V1 - V2 - V3 + bias
12 matmuls per 14 output rows per 128-channel half instead of the direct
conv's 18 -- 1.5x less PE work, and PE is the bottleneck.

Width padding is avoided entirely: the dw=1 (center) tap runs first at
full width N=392 with start=True, then the dw=0/2 taps accumulate into
column-shifted PSUM windows at N=385 -- the skipped edge column is
exactly the tap's zero-pad contribution.

Engine split (arrived at over ~10 traced iterations):
 - PE: 12 MMs per (group, half) at N=392; weights resident in SBUF, the
   LDWEIGHTS for each hides under the 166ns MM spacing (measured: warm
   steady-state spacing == the 392-cycle streaming floor, zero gaps).
 - DVE: the 4 unavoidable two-tensor output combines per (group, half)
   (max one PSUM operand each -- hw limit) + image 0's input transform
   (computed per 7-tile-row piece so MM #1 can start ~10.5us in).
 - GpSimd: input transform for images 1..3 (staged two images ahead so
   its ~3.8us/op rate stays off the critical path) + the weight/bias DMA
   issues (its queue is idle early; DMA issue ops cost ~650ns each and
   would serialize behind the image chunks on Sync).
 - ACT: bias folding (c0 = V0+b, c3 = b-V3, reading PSUM) + xs edge
   memsets.
 - Sync: image DMAs -- fully-contiguous 6272B/partition transfers into
   an H-padded-only slab (a W-padded slab forces 112B-chunk DMA at 1/4
   rate); image 0 is split into 4 row-chunks so the first transform
   piece gates on 229KB only; all output DMAs.
 - PE warmup: 36 dummy matmuls bridge the preamble+fill window so the
   HAM clock-gate (K=4/8 cold throttle) lifts before the real stream.
"""

import numpy as np
from contextlib import ExitStack

import concourse.bass as bass
import concourse.bacc as bacc
import concourse.mybir as mybir
import concourse.tile as tile
from concourse.bass_utils import run_bass_kernel_spmd

N_CORES = 8
N_BATCH = 32
N_PER_CORE = N_BATCH // N_CORES  # 4
C_IN = 128
C_OUT = 256
H = W = 56
HP = H + 2           # zero-padded height (in xs)
T_IMG = H // 2       # 28 tile rows per image
T_GRP = 7            # tile rows per matmul group
NGRP = T_IMG // T_GRP  # 4 groups -> 14 output rows each

f32 = mybir.dt.float32
bf16 = mybir.dt.bfloat16
f16 = mybir.dt.float16
AF = mybir.ActivationFunctionType


def build_program() -> bass.Bass:
    nc = bacc.Bacc("TRN2", target_bir_lowering=False, debug=False)
    x = nc.dram_tensor("x", [N_PER_CORE, C_IN, H, W], bf16, kind="ExternalInput")
    # wt[half, i, a, dw, o]: host-transformed Winograd weights. The values
    # (0, +-0.5, +-1, +-1.5) are exact in fp8-e4m3, which halves the
    # weight-stream bytes (the early-fill gate) at zero accuracy cost;
    # a single-fp8 lhsT against a bf16 rhs runs at bf16 speed.
    fp8 = mybir.dt.float8e4
    wt = nc.dram_tensor("wt", [2, C_IN, 4, 3, 128], fp8, kind="ExternalInput")
    b = nc.dram_tensor("b", [C_OUT], f32, kind="ExternalInput")
    # fp16 output: halves the 12.8MB/core output DMA (host upcasts to
    # f32; fp16 rounding of |y|<~190 adds ~2e-4 rel -- measured 3.0e-3
    # total, same as the f32-out version to 1 digit)
    y = nc.dram_tensor("y", [N_PER_CORE, C_OUT, H, W], f16, kind="ExternalOutput")

    with tile.TileContext(nc) as tc, ExitStack() as ctx:
        singles = ctx.enter_context(tc.tile_pool(name="singles", bufs=1))
        xsp = ctx.enter_context(tc.tile_pool(name="xsp", bufs=4))
        up = ctx.enter_context(tc.tile_pool(name="up", bufs=3))
        psum_mm = ctx.enter_context(
            tc.tile_pool(name="psum_mm", bufs=8, space="PSUM")
        )
        tdp = ctx.enter_context(tc.tile_pool(name="tdp", bufs=8))
        obp = ctx.enter_context(tc.tile_pool(name="obp", bufs=4))

        # warmup weight tile first: its GpSimd memset is the earliest
        # producer any PE work can gate on
        warm_w = singles.tile([128, 128], bf16)
        nc.gpsimd.memset(warm_w, 0.0)

        wtile = singles.tile([128, 2, 4, 3, 128], mybir.dt.float8e4,
                             name="wt")
        bsb = singles.tile([128, 2], f32)

        def stage_image(n):
            """One contiguous DMA into the H-padded slab + edge memsets +
            the four H-transform ops (GpSimd; staged 2 images ahead)."""
            xs = xsp.tile([128, HP, W], bf16, name="xs")
            nc.sync.dma_start(out=xs[:, 1:1 + H, :], in_=x.ap()[n])
            nc.scalar.memzero(xs[:, 0, :])
            nc.scalar.memzero(xs[:, HP - 1, :])

            def e(k):  # rows k, k+2, ..., k+54 of the padded slab
                return xs[:, k:k + 2 * (T_IMG - 1) + 1:2, :]

            U = [up.tile([128, T_IMG, W], bf16, name=f"u{a}", tag=f"u{a}")
                 for a in range(4)]
            nc.gpsimd.tensor_sub(U[0], e(0), e(2))
            nc.gpsimd.tensor_add(U[1], e(1), e(2))
            nc.gpsimd.tensor_sub(U[2], e(2), e(1))
            nc.gpsimd.tensor_sub(U[3], e(1), e(3))
            return U

        def stage_image0():
            """Image 0 rides the critical path: 4 row-chunk DMAs (first
            matmul gates on a 229KB transfer, not 784KB) + per-group
            U-transform pieces on the DVE; weights/bias issue from the
            GpSimd queue in parallel, (half0, a0) first."""
            xs = xsp.tile([128, HP, W], bf16, name="xs")
            # weights + bias issue from the (idle) GpSimd queue so their
            # ~650ns issue ops don't serialize behind the image chunks
            # on the Sync queue; (half0, a0) goes alone so MM #1's gate
            # is a 196KB transfer
            nc.gpsimd.dma_start(out=wtile[:, 0, 0], in_=wt.ap()[0][:, 0])
            nc.gpsimd.dma_start(out=wtile[:, 0, 1:4], in_=wt.ap()[0][:, 1:4])
            nc.gpsimd.dma_start(out=wtile[:, 1], in_=wt.ap()[1])
            nc.gpsimd.dma_start(
                out=bsb, in_=b.ap().rearrange("(h o) -> o h", h=2))
            chunks = [(0, 16), (16, 30), (30, 44), (44, 56)]
            for lo, hi in chunks:
                nc.sync.dma_start(out=xs[:, 1 + lo:1 + hi, :],
                                  in_=x.ap()[0, :, lo:hi, :])
            nc.scalar.memzero(xs[:, 0, :])
            nc.scalar.memzero(xs[:, HP - 1, :])

            U = [up.tile([128, T_IMG, W], bf16, name=f"u{a}", tag=f"u{a}")
                 for a in range(4)]

            def piece(g, eng):
                r = slice(T_GRP * g, T_GRP * (g + 1))

                def e(k):  # rows 14g+k, +2, ..., +12 of the padded slab
                    return xs[:, 14 * g + k:14 * g + k + 13:2, :]

                eng.tensor_sub(U[0][:, r, :], e(0), e(2))
                eng.tensor_add(U[1][:, r, :], e(1), e(2))
                eng.tensor_sub(U[2][:, r, :], e(2), e(1))
                eng.tensor_sub(U[3][:, r, :], e(1), e(3))

            # pieces 0-2 on DVE (fast, ahead of the output-op backlog),
            # the last on GpSimd (idle until image 1's transform; its
            # ~1us/op rate still beats group 3's ~21us deadline)
            for g in range(NGRP - 1):
                piece(g, nc.vector)
            piece(NGRP - 1, nc.gpsimd)
            return U

        # ---- PE warmup: bridge the pipeline-fill window (preamble + first
        # image DMA + first U transform) with dummy matmuls so the HAM
        # clock-gate lifts before the real stream starts.
        wp = psum_mm.tile([128, 128], f32, tag="ps")
        NWARM = 36
        for k in range(NWARM):
            nc.tensor.matmul(wp, lhsT=warm_w, rhs=warm_w,
                             start=(k == 0), stop=(k == NWARM - 1))

        def do_group(n, U, g, half, split_dma=False):
            """14 output rows (tile rows 7g..7g+6) of image n, one half."""
            h0 = 2 * T_GRP * g
            r = slice(T_GRP * g, T_GRP * (g + 1))
            V = [None] * 4
            # last group only: a=3 first, so its bias-fold (c3) runs during
            # the remaining matmuls and only y0/y1 trail the last one
            for a in ((3, 0, 1, 2) if split_dma else (0, 1, 2, 3)):
                ps = psum_mm.tile([128, T_GRP, W], f32, name=f"v{a}",
                                  tag="ps")
                lt = wtile[:, half, a]
                # center tap first at full width (sets has_written), then
                # the shifted taps accumulate into partial column windows
                nc.tensor.matmul(ps, lhsT=lt[:, 1], rhs=U[a][:, r, :],
                                 start=True, stop=False)
                nc.tensor.matmul(ps[:, :, 1:W], lhsT=lt[:, 0],
                                 rhs=U[a][:, r, 0:W - 1],
                                 start=False, stop=False)
                nc.tensor.matmul(ps[:, :, 0:W - 1], lhsT=lt[:, 2],
                                 rhs=U[a][:, r, 1:W],
                                 start=False, stop=True)
                V[a] = ps
            # y0 = V0+V1+V2+b, y1 = V1-V2-V3+b; ACT folds the bias into the
            # single-use terms, DVE does the two-tensor combines (max one
            # PSUM operand each).
            ob = obp.tile([128, T_GRP, 2, W], f16, name="ob", tag="ob")
            c0 = tdp.tile([128, T_GRP, W], f32, name="c0", tag="td")
            c3 = tdp.tile([128, T_GRP, W], f32, name="c3", tag="td")
            t = tdp.tile([128, T_GRP, W], f32, name="t", tag="td")
            e = tdp.tile([128, T_GRP, W], f32, name="e", tag="td")

            def act_c0():
                nc.scalar.activation(c0, V[0], AF.Identity,
                                     bias=bsb[:, half:half + 1])

            def act_c3():
                nc.scalar.activation(c3, V[3], AF.Identity,
                                     bias=bsb[:, half:half + 1], scale=-1.0)

            ych = y.ap()[n, half * 128:(half + 1) * 128]
            if split_dma:
                # tail: c3/c0/t/e all complete during the a=0..2 matmuls
                # (a=3 ran first), so only y0/y1 trail the last matmul --
                # and they go in two row-blocks so the first block's DMA
                # overlaps the second block's compute. All transfers stay
                # row-contiguous.
                act_c3(), act_c0()
                nc.vector.tensor_add(t, c0, V[1])
                nc.vector.tensor_add(e, c3, V[1])
                for rlo, rhi in ((0, 4), (4, T_GRP)):
                    nc.vector.tensor_add(ob[:, rlo:rhi, 0, :],
                                         t[:, rlo:rhi], V[2][:, rlo:rhi])
                    nc.vector.tensor_sub(ob[:, rlo:rhi, 1, :],
                                         e[:, rlo:rhi], V[2][:, rlo:rhi])
                    nc.sync.dma_start(
                        out=ych[:, h0 + 2 * rlo:h0 + 2 * rhi, :],
                        in_=ob[:, rlo:rhi],
                    )
            else:
                act_c0()
                nc.vector.tensor_add(t, c0, V[1])
                nc.vector.tensor_add(ob[:, :, 0, :], t, V[2])
                act_c3()
                nc.vector.tensor_add(e, c3, V[1])
                nc.vector.tensor_sub(ob[:, :, 1, :], e, V[2])
                nc.sync.dma_start(out=ych[:, h0:h0 + 2 * T_GRP, :], in_=ob)

        # software pipeline: staged two images ahead so GpSimd's slower
        # transform rate never gates the PE
        Us = [stage_image0(), stage_image(1), stage_image(2)]
        for n in range(N_PER_CORE):
            if n + 3 < N_PER_CORE:
                Us.append(stage_image(n + 3))
            for g in range(NGRP):
                for half in range(2):
                    last = (n == N_PER_CORE - 1 and g == NGRP - 1
                            and half == 1)
                    do_group(n, Us[n], g, half, split_dma=last)
    nc.compile()
    return nc


# F(2,3) weight transform G (exact in bf16 for +-1 weights)
_G = np.array([[1, 0, 0], [0.5, 0.5, 0.5], [0.5, -0.5, 0.5], [0, 0, 1]],
              dtype=np.float32)


def host_weight_layout(weight: np.ndarray) -> np.ndarray:
    """[256, 128, 3, 3] -> binarize, G-transform along dh,
    layout [half, i, a, dw, o] = [2, 128, 4, 3, 128] fp8-e4m3
    (values 0/+-0.5/+-1/+-1.5 are e4m3-exact)."""
    import ml_dtypes
    wc = np.clip(weight.astype(np.float32), -1.0, 1.0)
    wbin = np.where(wc >= 0, 1.0, -1.0).astype(np.float32)
    wtr = np.einsum("ad,oidw->aoiw", _G, wbin)     # [a, o, i, dw]
    w5 = wtr.reshape(4, 2, 128, C_IN, 3)           # [a, half, oo, i, dw]
    w6 = w5.transpose(1, 3, 0, 4, 2)               # [half, i, a, dw, oo]
    return np.ascontiguousarray(w6).astype(ml_dtypes.float8_e4m3fn)


def run(x, weight, bias, trace=False):
    """Returns (out [32,256,56,56] f32, BassKernelResults)."""
    import ml_dtypes
    nc = build_program()
    xb = np.asarray(x, dtype=np.float32).astype(ml_dtypes.bfloat16)
    wtr = host_weight_layout(np.asarray(weight))
    bias = np.ascontiguousarray(np.asarray(bias), dtype=np.float32)
    in_maps = [
        {
            "x": xb[i * N_PER_CORE:(i + 1) * N_PER_CORE],
            "wt": wtr,
            "b": bias,
        }
        for i in range(N_CORES)
    ]
    res = run_bass_kernel_spmd(
        nc, in_maps, core_ids=list(range(N_CORES)), trace=trace
    )
    out = np.concatenate([r["y"] for r in res.results], axis=0)
    return out.astype(np.float32), res


def kernel(x, weight, bias):
    out, _ = run(x, weight, bias)
    return out

